# revision 36
# baseline (speedup 1.0000x reference)
"""Trainium2 Bass kernel for causal multi-head attention block.

Reference computation (B=4, S=2048, D=1024, H=16, HD=64, fp32):
    qkv = x @ Wqkv + bqkv; split q,k,v; per-head scaled scores;
    causal mask filled with -0.0001 (leaky, NOT -inf); softmax over all
    2048 keys; out = P @ V; out = out @ Wo + bo.

Sharding (head-split tensor parallel): core i = (batch b = i//2,
head half p = i%2). Each core computes ALL 2048 queries of its batch
for heads 8p..8p+7: QKV projections column-sharded by head, attention
device-local, output projection row-sharded (contraction over this
core's 512 head-dims) -> partial outputs. The two partials per batch
are summed at unshard time (host gather). The V bias is absorbed into
the per-core output bias: out_head = P@V0 + bv exactly (softmax rows
sum to 1), so bv contributes bv @ Wo_mine.

Leaky-mask algebra (w = exp(-1e-4)):
  - scores per 512-query tile t against key blocks 0..t; the diagonal
    block is split so the fully-masked upper 256-key piece of the
    first query half is never computed.
  - masked chunks: S' = (S + 8e-4) * M fused on PSUM (one DVE op),
    then exp(0.125*S' - 1e-4) = exp(S/8) unmasked / w masked.
  - skipped key blocks contribute w*Suf[d] to the numerator (suffix
    sums of unbiased V at 256-block granularity) and w*nskip to Z.
Z comes from a 65th all-ones V column in the PV matmul; 1/Z is
broadcast across the 64 head-dims with a rank-1 PE matmul.

Scheduling: the scores->exp->PV chain is software-pipelined (lag 1
chunk); tile epilogues are deferred into the next tile's chunk stream
so the in-order PE queue never blocks on the DVE z-chain; projection
work (Q/K per pair, V per head-group, wo convert, output projection
per query chunk) is sliced into tasks popped between attention chunks
as PE filler, with label gating for emission-order correctness. This
keeps the PE stream dense so the HAM clock stays at 2.4 GHz.
"""

import math
from contextlib import ExitStack

import numpy as np

import concourse.bass as bass
import concourse.mybir as mybir
import concourse.tile as tile
from concourse import bacc

F32 = mybir.dt.float32
F32R = mybir.dt.float32r
BF16 = mybir.dt.bfloat16
AF = mybir.ActivationFunctionType
ALU = mybir.AluOpType
AX = mybir.AxisListType

B, S, D, H, HD = 4, 2048, 1024, 16, 64
HPC = 8            # heads per core
NP = 4             # head pairs per core
NCH = D // 128     # contraction chunks
NT = 4             # 512-query tiles
W_MASK = math.exp(-1e-4)


def build_program():
    nc = bacc.Bacc(
        "TRN2",
        target_bir_lowering=False,
        debug=False,
        num_devices=8,
    )
    xT = nc.declare_dram_parameter("xT", [D, S], BF16, isOutput=False)
    xsum = nc.declare_dram_parameter("xsum", [128, NCH, 8], BF16, isOutput=False)
    wq = nc.declare_dram_parameter("wq", [D, 512], BF16, isOutput=False)
    wk = nc.declare_dram_parameter("wk", [D, 512], BF16, isOutput=False)
    wv = nc.declare_dram_parameter("wv", [D, 512], BF16, isOutput=False)
    wo = nc.declare_dram_parameter("wo", [512, D], F32, isOutput=False)
    bqk = nc.declare_dram_parameter("bqk", [128, 8], F32, isOutput=False)
    bocol = nc.declare_dram_parameter("bocol", [128, 8], F32, isOutput=False)
    mmul = nc.declare_dram_parameter("mmul", [128, 2, 768], BF16, isOutput=False)
    zcrow = nc.declare_dram_parameter("zcrow", [1, S], F32, isOutput=False)
    wmadd = nc.declare_dram_parameter("wmadd", [128, 2, 256], F32, isOutput=False)
    outT = nc.declare_dram_parameter("outT", [D, S], F32, isOutput=True)

    with tile.TileContext(nc) as tc, ExitStack() as ctx, \
         nc.allow_low_precision(reason="float32r matmul inputs are fp32 bits"):
        consts = ctx.enter_context(tc.tile_pool(name="consts", bufs=1))
        bqk_sb = consts.tile([128, 8], F32)
        nc.sync.dma_start(out=bqk_sb, in_=bqk[:])
        bocol_sb = consts.tile([128, 8], F32)
        nc.sync.dma_start(out=bocol_sb, in_=bocol[:])
        mmul_sb = consts.tile([128, 2, 768], BF16)
        nc.sync.dma_start(out=mmul_sb, in_=mmul[:])
        zcrow_sb = consts.tile([1, S], F32)
        nc.sync.dma_start(out=zcrow_sb, in_=zcrow[0:1, :])
        wmadd_sb = consts.tile([128, 2, 256], F32)
        nc.sync.dma_start(out=wmadd_sb, in_=wmadd[:])
        xsum_sb = consts.tile([128, NCH, 8], BF16)
        nc.sync.dma_start(out=xsum_sb, in_=xsum[:])
        onef = consts.tile([128, 128], F32)
        nc.vector.memset(onef, 1.0)
        ones_fr = consts.tile([1, 64], F32R)
        nc.vector.tensor_copy(out=ones_fr, in_=onef[0:1, 0:64])
        bias_neg = consts.tile([128, 1], F32)
        nc.vector.memset(bias_neg, -1e-4)
        warm = consts.tile([1, 16], F32)
        nc.scalar.activation(out=warm, in_=onef[0:1, 0:16], func=AF.Exp)

        wv_pool = ctx.enter_context(tc.tile_pool(name="wvp", bufs=1))
        wv_sb = wv_pool.tile([128, NCH, 512], BF16)
        nc.sync.dma_start(
            out=wv_sb, in_=wv[:].rearrange("(c p) m -> p c m", p=128))

        xt_pool = ctx.enter_context(tc.tile_pool(name="xt", bufs=1))
        xT_sb = xt_pool.tile([128, NCH, S], BF16)
        for c in range(NCH):
            nc.sync.dma_start(out=xT_sb[:, c, :], in_=xT[128 * c:128 * (c + 1), :])

        # persistent attention-side tensors
        big = ctx.enter_context(tc.tile_pool(name="big", bufs=1))
        V_sb = big.tile([128, 16, HPC, 65], F32R)  # [key sub, tok blk, head, d+1]
        O_sb = big.tile([128, NP, S], BF16)        # [2 heads x 64, chunk(=pair), q]
        suf_sb = big.tile([64, NP, 2, 9], F32)     # [d, pair, head, 256-block idx]
        wo_bf = big.tile([128, NP, 8, 128], BF16)
        nc.vector.tensor_copy(
            out=V_sb[:, :, :, 64],
            in_=onef.rearrange("p (a b) -> p a b", a=16)[:, :, 0:8])

        qk_ring = ctx.enter_context(tc.tile_pool(name="qkr", bufs=2))
        w_ring = ctx.enter_context(tc.tile_pool(name="wr", bufs=2))
        wof = ctx.enter_context(tc.tile_pool(name="wof", bufs=1))

        qt_tiles = {}
        kt_tiles = {}
        w_tiles = {}
        misc_holder = {}

        # ================= task machinery =================
        # Each task: (label, closure(pool)). Tasks emit PE work into the
        # given PSUM pool via tag "pj". Label gating guarantees emission-
        # order correctness; surplus tasks are popped one per chunk as PE
        # filler so the PE stream stays dense.
        def v_task(t):
            def run(pool):
                ps = pool.tile([128, 512], F32, tag=pool._pjtag, name="psv", bufs=pool._pjbufs)
                for c in range(NCH):
                    nc.tensor.matmul(
                        out=ps, lhsT=xT_sb[:, c, 128 * t:128 * (t + 1)],
                        rhs=wv_sb[:, c, :],
                        start=(c == 0), stop=(c == NCH - 1),
                    )
                nc.vector.tensor_copy(
                    out=V_sb[:, t, :, 0:64],
                    in_=ps.rearrange("p (h d) -> p h d", h=8),
                )
            return (("v", t), run)

        def qk_task(which, pr, qc):
            def run(pool):
                if qc == 0:
                    w_tiles[(which, pr)] = w_ring.tile(
                        [128, NCH, 128], BF16, tag=which,
                        name=f"w_{which}{pr}")
                    src = wq if which == "q" else wk
                    nc.sync.dma_start(
                        out=w_tiles[(which, pr)],
                        in_=src[:, 128 * pr:128 * (pr + 1)].rearrange(
                            "(c p) m -> p c m", p=128))
                    dst = qk_ring.tile([128, S], BF16, tag=which,
                                       name=f"qk_{which}{pr}")
                    if which == "q":
                        qt_tiles[pr] = dst
                    else:
                        kt_tiles[pr] = dst
                w_sb = w_tiles[(which, pr)]
                dst = qt_tiles[pr] if which == "q" else kt_tiles[pr]
                bcol = pr if which == "q" else 4 + pr
                ps = pool.tile([128, 512], F32, tag=pool._pjtag, name="psqk", bufs=pool._pjbufs)
                for c in range(NCH):
                    nc.tensor.matmul(
                        out=ps, lhsT=w_sb[:, c, :],
                        rhs=xT_sb[:, c, 512 * qc:512 * (qc + 1)],
                        start=(c == 0), stop=(c == NCH - 1),
                    )
                nc.vector.tensor_scalar_add(
                    out=dst[:, 512 * qc:512 * (qc + 1)], in0=ps,
                    scalar1=bqk_sb[:, bcol:bcol + 1],
                )
            return ((which, pr, qc), run)

        def bs_task(pr):
            def run(pool):
                psb = pool.tile([128, 8], F32, tag=pool._pjtag, name="psb", bufs=pool._pjbufs)
                for c in range(NCH):
                    nc.tensor.matmul(
                        out=psb, lhsT=wv_sb[:, c, 128 * pr:128 * (pr + 1)],
                        rhs=xsum_sb[:, c, :],
                        start=(c == 0), stop=(c == NCH - 1),
                    )
                for hl in range(2):
                    nc.vector.memset(suf_sb[:, pr, hl, 8:9], 0.0)
                    for i in range(7, -1, -1):
                        nc.vector.scalar_tensor_tensor(
                            out=suf_sb[:, pr, hl, i:i + 1],
                            in0=psb[64 * hl:64 * hl + 64, i:i + 1],
                            scalar=W_MASK, in1=suf_sb[:, pr, hl, i + 1:i + 2],
                            op0=ALU.mult, op1=ALU.add,
                        )
            return (("bs", pr), run)

        def wo_task(hc):
            def run(pool):
                wo_f = wof.tile([128, 8, 128], F32, tag="wof", name="wo_f")
                nc.sync.dma_start(
                    out=wo_f,
                    in_=wo[128 * hc:128 * (hc + 1), :].rearrange(
                        "p (d m) -> p d m", m=128))
                nc.vector.tensor_copy(out=wo_bf[:, hc, :, :], in_=wo_f)
            return (("wo", hc), run)

        def oproj_task(dc, qc):
            def run(pool):
                ps = pool.tile([128, 512], F32, tag=pool._pjtag, name="psop", bufs=pool._pjbufs)
                for hc in range(NP):
                    nc.tensor.matmul(
                        out=ps, lhsT=wo_bf[:, hc, dc, :],
                        rhs=O_sb[:, hc, 512 * qc:512 * (qc + 1)],
                        start=(hc == 0), stop=(hc == NP - 1),
                    )
                fo = misc_holder["misc"].tile([128, 512], F32, tag="fo")
                nc.vector.tensor_scalar_add(
                    out=fo, in0=ps, scalar1=bocol_sb[:, dc:dc + 1])
                nc.sync.dma_start(
                    out=outT[128 * dc:128 * (dc + 1), 512 * qc:512 * (qc + 1)],
                    in_=fo,
                )
            return (("op", dc, qc), run)

        taskq = []
        done_labels = set()

        def pop_one(pool):
            if taskq:
                label, run = taskq.pop(0)
                run(pool)
                done_labels.add(label)

        def ensure(labels, pool):
            while taskq and not all(l in done_labels for l in labels):
                pop_one(pool)

        def pop_toward(labels, pool):
            if taskq and not all(l in done_labels for l in labels):
                pop_one(pool)
            elif taskq and not labels:
                pop_one(pool)

        # ---------------- upfront minimal (own PSUM scope) ----------------
        with tc.tile_pool(name="ups", bufs=4, space="PSUM") as ups:
            ups._pjtag = "pj"
            ups._pjbufs = 4
            # dummy matmuls fill the input-DMA window so the PE HAM clock
            # warms to 2.4 GHz before real work arrives
            wps = ups.tile([128, 128], F32, tag="warm", bufs=2)
            for i in range(64):
                nc.tensor.matmul(
                    out=wps, lhsT=onef, rhs=onef,
                    start=(i == 0), stop=(i == 63), skip_group_check=True)
            for t in range(4):
                v_task(t)[1](ups)
            qk_task("q", 0, 0)[1](ups)
            qk_task("k", 0, 0)[1](ups)
        done_labels.update({("v", t) for t in range(4)})
        done_labels.update({("q", 0, 0), ("k", 0, 0)})

        # remaining work as ordered tasks
        taskq.append(bs_task(0))
        for qc in range(1, NT):
            taskq.append(qk_task("k", 0, qc))
            taskq.append(qk_task("q", 0, qc))
            for t in range(4 * qc, 4 * qc + 4):
                taskq.append(v_task(t))
        for qc in range(NT):
            taskq.append(qk_task("q", 1, qc))
            taskq.append(qk_task("k", 1, qc))
        taskq.append(bs_task(1))
        for hc in range(NP):
            taskq.append(wo_task(hc))
        for pr in range(2, NP):
            for qc in range(NT):
                taskq.append(qk_task("q", pr, qc))
                taskq.append(qk_task("k", pr, qc))
            taskq.append(bs_task(pr))

        # ---------------- attention ----------------
        with tc.tile_pool(name="sps", bufs=2, space="PSUM") as sps, \
             tc.tile_pool(name="pop", bufs=2, space="PSUM") as pop, \
             tc.tile_pool(name="epool", bufs=5) as epool, \
             tc.tile_pool(name="misc", bufs=2) as misc:
            sps._pjtag = "pj"
            sps._pjbufs = 2
            misc_holder["misc"] = misc

            deferred = []   # epilogue-PE + nm/ot closures of previous tile

            def flush_deferred():
                while deferred:
                    deferred.pop(0)()

            def tile_need(pr, t):
                return ([("q", pr, t)] + [("k", pr, kc) for kc in range(t + 1)]
                        + [("v", tb) for tb in range(4 * t + 4)]
                        + [("bs", pr)])

            for pr in range(NP):
                ensure([("q", pr, 0), ("k", pr, 0)]
                       + [("v", t) for t in range(4)], sps)
                QT = qt_tiles[pr]
                KT = kt_tiles[pr]
                for t in range(NT):
                    ensure(tile_need(pr, t), sps)
                    nxt = (tile_need(pr, t + 1) if t + 1 < NT
                           else (tile_need(pr + 1, 0) if pr + 1 < NP else []))
                    q0 = 512 * t
                    chunks = []
                    for kb in range(t):
                        for s2 in range(4):
                            chunks.append((512 * kb + 128 * s2, q0, 512, None))
                    for s2 in range(2):
                        chunks.append((q0 + 128 * s2, q0, 512, ("A", s2)))
                    for s2 in range(2):
                        chunks.append((q0 + 256 + 128 * s2, q0 + 256, 256,
                                       ("B", s2)))

                    po = [pop.tile([65, 512], F32, tag="po", name=f"po{hl}")
                          for hl in range(2)]
                    ntot = len(chunks)
                    pend = []

                    def emit_scores(ci, chunks=chunks, pend=pend, QT=QT, KT=KT):
                        ko, qlo, qn, _ = chunks[ci]
                        pt = sps.tile([128, 2, 512], F32, tag="s", name="pt", bufs=2)
                        for hl in range(2):
                            hs = slice(64 * hl, 64 * (hl + 1))
                            nc.tensor.matmul(
                                out=pt[:, hl, 0:qn],
                                lhsT=KT[hs, ko:ko + 128],
                                rhs=QT[hs, qlo:qlo + qn],
                                start=True, stop=True,
                            )
                        pend.append((ci, pt))

                    def emit_exp(ci, pt, chunks=chunks):
                        # uniform bias -1e-4 scales every softmax term by w
                        # (cancels in the normalization); masked entries then
                        # come out as w (vs exact w^2) -- a 1e-4 relative
                        # perturbation of those weights, far below tolerance.
                        ko, qlo, qn, mr = chunks[ci]
                        if mr is not None:
                            # S' = S*M pre-exp; with the uniform -1e-4 exp
                            # bias, masked lanes come out as w
                            _, s2 = mr
                            nc.vector.tensor_mul(
                                out=pt[:, :, 0:256], in0=pt[:, :, 0:256],
                                in1=mmul_sb[:, s2:s2 + 1, 0:256].broadcast_to(
                                    [128, 2, 256]),
                            )
                        e = epool.tile([128, 2, 512], F32R, tag="e")
                        nc.scalar.activation(
                            out=e[:, :, 0:qn], in_=pt[:, :, 0:qn],
                            func=AF.Exp, scale=0.125, bias=bias_neg[:, 0:1],
                        )
                        return e

                    def emit_pv(ci, e, chunks=chunks, po=po, ntot=ntot,
                                q0=q0, pr=pr):
                        ko, qlo, qn, _ = chunks[ci]
                        qrel = qlo - q0
                        for hl in range(2):
                            nc.tensor.matmul(
                                out=po[hl][:, qrel:qrel + qn],
                                lhsT=V_sb[:, ko // 128, 2 * pr + hl, :],
                                rhs=e[:, hl, 0:qn],
                                start=(ci == 0),
                                stop=(ci == ntot - 1),
                                skip_group_check=True,
                            )

                    # software-pipelined emission (lag 1 chunk); previous
                    # tile's deferred epilogue flushes before our first PV
                    # (its po buffers are about to be reused)
                    epend = []
                    for ci in range(ntot):
                        emit_scores(ci)
                        if ci == 1:
                            flush_deferred()
                        if len(pend) >= 2:
                            cj, pt = pend.pop(0)
                            epend.append((cj, emit_exp(cj, pt)))
                        if len(epend) >= 2:
                            cj, e = epend.pop(0)
                            emit_pv(cj, e)
                            if cj != 0:
                                pop_one(sps)
                    while pend:
                        cj, pt = pend.pop(0)
                        epend.append((cj, emit_exp(cj, pt)))
                    while epend:
                        cj, e = epend.pop(0)
                        emit_pv(cj, e)

                    # z-chain now (DVE only); PE parts + nm/ot deferred
                    zrow = misc.tile([1, 1024], F32, tag="zrow", bufs=2)
                    for hl in range(2):
                        nc.vector.scalar_tensor_tensor(
                            out=zrow[:, 512 * hl:512 * (hl + 1)],
                            in0=po[hl][64:65, 0:512], scalar=0.0,
                            in1=zcrow_sb[0:1, 512 * t:512 * (t + 1)],
                            op0=ALU.add, op1=ALU.add,
                        )
                    nc.vector.reciprocal_approx_fast(out=zrow, in_=zrow)
                    zr = misc.tile([1, 1024], F32R, tag="zr", bufs=2)
                    nc.vector.tensor_copy(out=zr, in_=zrow)

                    def late(po=po, zr=zr, pr=pr, t=t, q0=q0):
                        for hl in range(2):
                            zb = sps.tile([64, 512], F32, tag="s", name="zb", bufs=2)
                            nc.tensor.matmul(
                                out=zb, lhsT=ones_fr,
                                rhs=zr[0:1, 512 * hl:512 * (hl + 1)],
                                start=True, stop=True, skip_group_check=True,
                            )
                            nm = misc.tile([64, 512], F32, tag="nm")
                            for qh in range(2):
                                nc.vector.tensor_scalar_add(
                                    out=nm[:, 256 * qh:256 * (qh + 1)],
                                    in0=po[hl][0:64, 256 * qh:256 * (qh + 1)],
                                    scalar1=suf_sb[:, pr, hl,
                                                   2 * t + 1 + qh:2 * t + 2 + qh],
                                )
                            nc.vector.tensor_mul(
                                out=O_sb[64 * hl:64 * (hl + 1), pr, q0:q0 + 512],
                                in0=nm, in1=zb,
                            )
                    deferred.append(late)
                    if pr == NP - 1:
                        # output projection for query chunk t becomes
                        # available once this (last) pair's tile t is done
                        def oq(t=t):
                            for dc in range(8):
                                taskq.append(oproj_task(dc, t))
                        deferred.append(oq)

            flush_deferred()
        with tc.tile_pool(name="fin", bufs=4, space="PSUM") as fin, \
             tc.tile_pool(name="fmisc", bufs=4) as fmisc:
            fin._pjtag = "pj"
            fin._pjbufs = 4
            misc_holder["misc"] = fmisc
            while taskq:
                pop_one(fin)
    nc.compile()
    return nc


def host_in_maps(x, Wqkv, bqkv, Wo, bo):
    import ml_dtypes
    x = np.asarray(x, np.float32)
    Wqkv = np.ascontiguousarray(np.asarray(Wqkv, np.float32))
    bqkv = np.asarray(bqkv, np.float32)
    Wo = np.ascontiguousarray(np.asarray(Wo, np.float32))
    bo = np.asarray(bo, np.float32)

    # triangular 256x256 mask for 2 key sub-blocks, plus all-ones pad
    kap = np.arange(128)[:, None]
    r = np.arange(256)[None, :]
    tri = np.zeros((128, 2, 256), np.float32)
    for s2 in range(2):
        tri[:, s2, :] = (128 * s2 + kap <= r)
    mmul = np.ones((128, 2, 768), np.float32)
    mmul[:, :, 0:256] = tri
    mmul[:, :, 512:768] = tri
    mmul = np.ascontiguousarray(mmul.astype(ml_dtypes.bfloat16))

    wmadd_h = np.ascontiguousarray((1.0 - tri) * W_MASK)

    zcrow = np.zeros((1, S), np.float32)
    for t in range(NT):
        zcrow[0, 512 * t:512 * t + 256] = W_MASK * (S - 512 * t - 256)
        zcrow[0, 512 * t + 256:512 * t + 512] = W_MASK * (S - 512 * t - 512)

    xTs = []
    xsums = []
    for b in range(B):
        xt = np.ascontiguousarray(x[b].T)                # [D, S]
        xTs.append(np.ascontiguousarray(xt.astype(ml_dtypes.bfloat16)))
        xs = xt.reshape(NCH, 128, 8, 256).sum(axis=3)    # [c, p, blk]
        xsums.append(np.ascontiguousarray(
            xs.transpose(1, 0, 2).astype(ml_dtypes.bfloat16)))
    per_p = {}
    for p in range(2):
        cs = slice(512 * p, 512 * p + 512)
        bq = bqkv[0:D][cs]
        bk = bqkv[D:2 * D][cs]
        bv = bqkv[2 * D:][cs]
        wo_p = np.ascontiguousarray(Wo[cs, :])
        bqk = np.zeros((128, 8), np.float32)
        for pr in range(NP):
            bqk[:, pr] = bq[128 * pr:128 * (pr + 1)]
            bqk[:, 4 + pr] = bk[128 * pr:128 * (pr + 1)]
        boc = bv @ wo_p + (bo if p == 0 else 0.0)
        bocol = np.ascontiguousarray(boc.reshape(8, 128).T)
        per_p[p] = {
            "wq": np.ascontiguousarray(Wqkv[:, cs].astype(ml_dtypes.bfloat16)),
            "wk": np.ascontiguousarray(
                Wqkv[:, D + 512 * p:D + 512 * p + 512].astype(ml_dtypes.bfloat16)),
            "wv": np.ascontiguousarray(
                Wqkv[:, 2 * D + 512 * p:2 * D + 512 * p + 512].astype(
                    ml_dtypes.bfloat16)),
            "wo": wo_p,
            "bqk": bqk,
            "bocol": bocol,
        }

    in_maps = []
    for core in range(8):
        b, p = core // 2, core % 2
        m = {"xT": xTs[b], "xsum": xsums[b], "mmul": mmul, "zcrow": zcrow, "wmadd": wmadd_h}
        m.update(per_p[p])
        in_maps.append(m)
    return in_maps


def assemble(results):
    out = np.zeros((B, S, D), np.float32)
    for b in range(B):
        out[b] = (results[2 * b]["outT"] + results[2 * b + 1]["outT"]).T
    return out


_CACHED = {}


def get_program():
    if "nc" not in _CACHED:
        _CACHED["nc"] = build_program()
    return _CACHED["nc"]


def kernel(x, Wqkv, bqkv, Wo, bo):
    from concourse.bass_utils import run_bass_kernel_spmd

    nc = get_program()
    in_maps = host_in_maps(x, Wqkv, bqkv, Wo, bo)
    res = run_bass_kernel_spmd(nc, in_maps, core_ids=list(range(8)))
    return assemble(res.results)


# revision 37
# speedup vs baseline: 1.0157x; 1.0157x over previous
"""Trainium2 Bass kernel for causal multi-head attention block.

Reference computation (B=4, S=2048, D=1024, H=16, HD=64, fp32):
    qkv = x @ Wqkv + bqkv; split q,k,v; per-head scaled scores;
    causal mask filled with -0.0001 (leaky, NOT -inf); softmax over all
    2048 keys; out = P @ V; out = out @ Wo + bo.

Sharding (head-split tensor parallel): core i = (batch b = i//2,
head half p = i%2). Each core computes ALL 2048 queries of its batch
for heads 8p..8p+7: QKV projections column-sharded by head, attention
device-local, output projection row-sharded (contraction over this
core's 512 head-dims) -> partial outputs. The two partials per batch
are summed at unshard time (host gather). The V bias is absorbed into
the per-core output bias: out_head = P@V0 + bv exactly (softmax rows
sum to 1), so bv contributes bv @ Wo_mine.

Leaky-mask algebra (w = exp(-1e-4)):
  - scores per 512-query tile t against key blocks 0..t; the diagonal
    block is split so the fully-masked upper 256-key piece of the
    first query half is never computed.
  - masked chunks: S' = (S + 8e-4) * M fused on PSUM (one DVE op),
    then exp(0.125*S' - 1e-4) = exp(S/8) unmasked / w masked.
  - skipped key blocks contribute w*Suf[d] to the numerator (suffix
    sums of unbiased V at 256-block granularity) and w*nskip to Z.
Z comes from a 65th all-ones V column in the PV matmul; 1/Z is
broadcast across the 64 head-dims with a rank-1 PE matmul.

Scheduling: the scores->exp->PV chain is software-pipelined (lag 1
chunk); tile epilogues are deferred into the next tile's chunk stream
so the in-order PE queue never blocks on the DVE z-chain; projection
work (Q/K per pair, V per head-group, wo convert, output projection
per query chunk) is sliced into tasks popped between attention chunks
as PE filler, with label gating for emission-order correctness. This
keeps the PE stream dense so the HAM clock stays at 2.4 GHz.
"""

import math
from contextlib import ExitStack

import numpy as np

import concourse.bass as bass
import concourse.mybir as mybir
import concourse.tile as tile
from concourse import bacc

F32 = mybir.dt.float32
F32R = mybir.dt.float32r
BF16 = mybir.dt.bfloat16
AF = mybir.ActivationFunctionType
ALU = mybir.AluOpType
AX = mybir.AxisListType

B, S, D, H, HD = 4, 2048, 1024, 16, 64
HPC = 8            # heads per core
NP = 4             # head pairs per core
NCH = D // 128     # contraction chunks
NT = 4             # 512-query tiles
W_MASK = math.exp(-1e-4)


def build_program():
    nc = bacc.Bacc(
        "TRN2",
        target_bir_lowering=False,
        debug=False,
        num_devices=8,
    )
    xT = nc.declare_dram_parameter("xT", [D, S], BF16, isOutput=False)
    xsum = nc.declare_dram_parameter("xsum", [128, NCH, 8], BF16, isOutput=False)
    wq = nc.declare_dram_parameter("wq", [D, 512], BF16, isOutput=False)
    wk = nc.declare_dram_parameter("wk", [D, 512], BF16, isOutput=False)
    wv = nc.declare_dram_parameter("wv", [D, 512], BF16, isOutput=False)
    wo = nc.declare_dram_parameter("wo", [512, D], F32, isOutput=False)
    bqk = nc.declare_dram_parameter("bqk", [128, 8], F32, isOutput=False)
    bocol = nc.declare_dram_parameter("bocol", [128, 8], F32, isOutput=False)
    mmul = nc.declare_dram_parameter("mmul", [128, 2, 768], BF16, isOutput=False)
    zcrow = nc.declare_dram_parameter("zcrow", [1, S], F32, isOutput=False)
    wmadd = nc.declare_dram_parameter("wmadd", [128, 2, 256], F32, isOutput=False)
    outT = nc.declare_dram_parameter("outT", [D, S], F32, isOutput=True)

    with tile.TileContext(nc) as tc, ExitStack() as ctx, \
         nc.allow_low_precision(reason="float32r matmul inputs are fp32 bits"):
        consts = ctx.enter_context(tc.tile_pool(name="consts", bufs=1))
        bqk_sb = consts.tile([128, 8], F32)
        nc.sync.dma_start(out=bqk_sb, in_=bqk[:])
        bocol_sb = consts.tile([128, 8], F32)
        nc.sync.dma_start(out=bocol_sb, in_=bocol[:])
        mmul_sb = consts.tile([128, 2, 768], BF16)
        nc.sync.dma_start(out=mmul_sb, in_=mmul[:])
        zcrow_sb = consts.tile([1, S], F32)
        nc.sync.dma_start(out=zcrow_sb, in_=zcrow[0:1, :])
        wmadd_sb = consts.tile([128, 2, 256], F32)
        nc.sync.dma_start(out=wmadd_sb, in_=wmadd[:])
        xsum_sb = consts.tile([128, NCH, 8], BF16)
        nc.sync.dma_start(out=xsum_sb, in_=xsum[:])
        onef = consts.tile([128, 128], F32)
        nc.vector.memset(onef, 1.0)
        ones_fr = consts.tile([1, 64], F32R)
        nc.vector.tensor_copy(out=ones_fr, in_=onef[0:1, 0:64])
        bias_neg = consts.tile([128, 1], F32)
        nc.vector.memset(bias_neg, -1e-4)
        warm = consts.tile([1, 16], F32)
        nc.scalar.activation(out=warm, in_=onef[0:1, 0:16], func=AF.Exp)

        wv_pool = ctx.enter_context(tc.tile_pool(name="wvp", bufs=1))
        wv_sb = wv_pool.tile([128, NCH, 512], BF16)
        nc.sync.dma_start(
            out=wv_sb, in_=wv[:].rearrange("(c p) m -> p c m", p=128))

        xt_pool = ctx.enter_context(tc.tile_pool(name="xt", bufs=1))
        xT_sb = xt_pool.tile([128, NCH, S], BF16)
        for c in range(NCH):
            nc.sync.dma_start(out=xT_sb[:, c, :], in_=xT[128 * c:128 * (c + 1), :])

        # persistent attention-side tensors
        big = ctx.enter_context(tc.tile_pool(name="big", bufs=1))
        V_sb = big.tile([128, 16, HPC, 65], F32R)  # [key sub, tok blk, head, d+1]
        O_sb = big.tile([128, NP, S], BF16)        # [2 heads x 64, chunk(=pair), q]
        suf_sb = big.tile([64, NP, 2, 9], F32)     # [d, pair, head, 256-block idx]
        wo_bf = big.tile([128, NP, 8, 128], BF16)
        nc.vector.tensor_copy(
            out=V_sb[:, :, :, 64],
            in_=onef.rearrange("p (a b) -> p a b", a=16)[:, :, 0:8])

        qk_ring = ctx.enter_context(tc.tile_pool(name="qkr", bufs=2))
        w_ring = ctx.enter_context(tc.tile_pool(name="wr", bufs=2))
        wof = ctx.enter_context(tc.tile_pool(name="wof", bufs=1))

        qt_tiles = {}
        kt_tiles = {}
        w_tiles = {}
        misc_holder = {}

        # ================= task machinery =================
        # Each task: (label, closure(pool)). Tasks emit PE work into the
        # given PSUM pool via tag "pj". Label gating guarantees emission-
        # order correctness; surplus tasks are popped one per chunk as PE
        # filler so the PE stream stays dense.
        def v_task(t):
            def run(pool):
                ps = pool.tile([128, 512], F32, tag=pool._pjtag, name="psv", bufs=pool._pjbufs)
                for c in range(NCH):
                    nc.tensor.matmul(
                        out=ps, lhsT=xT_sb[:, c, 128 * t:128 * (t + 1)],
                        rhs=wv_sb[:, c, :],
                        start=(c == 0), stop=(c == NCH - 1),
                    )
                nc.vector.tensor_copy(
                    out=V_sb[:, t, :, 0:64],
                    in_=ps.rearrange("p (h d) -> p h d", h=8),
                )
            return (("v", t), run)

        def qk_task(which, pr, qc):
            def run(pool):
                if qc == 0:
                    w_tiles[(which, pr)] = w_ring.tile(
                        [128, NCH, 128], BF16, tag=which,
                        name=f"w_{which}{pr}")
                    src = wq if which == "q" else wk
                    nc.sync.dma_start(
                        out=w_tiles[(which, pr)],
                        in_=src[:, 128 * pr:128 * (pr + 1)].rearrange(
                            "(c p) m -> p c m", p=128))
                    dst = qk_ring.tile([128, S], BF16, tag=which,
                                       name=f"qk_{which}{pr}")
                    if which == "q":
                        qt_tiles[pr] = dst
                    else:
                        kt_tiles[pr] = dst
                w_sb = w_tiles[(which, pr)]
                dst = qt_tiles[pr] if which == "q" else kt_tiles[pr]
                bcol = pr if which == "q" else 4 + pr
                ps = pool.tile([128, 512], F32, tag=pool._pjtag, name="psqk", bufs=pool._pjbufs)
                for c in range(NCH):
                    nc.tensor.matmul(
                        out=ps, lhsT=w_sb[:, c, :],
                        rhs=xT_sb[:, c, 512 * qc:512 * (qc + 1)],
                        start=(c == 0), stop=(c == NCH - 1),
                    )
                nc.vector.tensor_scalar_add(
                    out=dst[:, 512 * qc:512 * (qc + 1)], in0=ps,
                    scalar1=bqk_sb[:, bcol:bcol + 1],
                )
            return ((which, pr, qc), run)

        def bs_task(pr):
            def run(pool):
                psb = pool.tile([128, 8], F32, tag=pool._pjtag, name="psb", bufs=pool._pjbufs)
                for c in range(NCH):
                    nc.tensor.matmul(
                        out=psb, lhsT=wv_sb[:, c, 128 * pr:128 * (pr + 1)],
                        rhs=xsum_sb[:, c, :],
                        start=(c == 0), stop=(c == NCH - 1),
                    )
                for hl in range(2):
                    nc.vector.memset(suf_sb[:, pr, hl, 8:9], 0.0)
                    for i in range(7, -1, -1):
                        nc.vector.scalar_tensor_tensor(
                            out=suf_sb[:, pr, hl, i:i + 1],
                            in0=psb[64 * hl:64 * hl + 64, i:i + 1],
                            scalar=W_MASK, in1=suf_sb[:, pr, hl, i + 1:i + 2],
                            op0=ALU.mult, op1=ALU.add,
                        )
            return (("bs", pr), run)

        def wo_task(hc):
            def run(pool):
                wo_f = wof.tile([128, 8, 128], F32, tag="wof", name="wo_f")
                nc.sync.dma_start(
                    out=wo_f,
                    in_=wo[128 * hc:128 * (hc + 1), :].rearrange(
                        "p (d m) -> p d m", m=128))
                nc.vector.tensor_copy(out=wo_bf[:, hc, :, :], in_=wo_f)
            return (("wo", hc), run)

        def oproj_task(dc, qc):
            def run(pool):
                ps = pool.tile([128, 512], F32, tag=pool._pjtag, name="psop", bufs=pool._pjbufs)
                for hc in range(NP):
                    nc.tensor.matmul(
                        out=ps, lhsT=wo_bf[:, hc, dc, :],
                        rhs=O_sb[:, hc, 512 * qc:512 * (qc + 1)],
                        start=(hc == 0), stop=(hc == NP - 1),
                    )
                fo = misc_holder["misc"].tile([128, 512], F32, tag="fo")
                nc.vector.tensor_scalar_add(
                    out=fo, in0=ps, scalar1=bocol_sb[:, dc:dc + 1])
                nc.sync.dma_start(
                    out=outT[128 * dc:128 * (dc + 1), 512 * qc:512 * (qc + 1)],
                    in_=fo,
                )
            return (("op", dc, qc), run)

        taskq = []
        done_labels = set()

        def pop_one(pool):
            if taskq:
                label, run = taskq.pop(0)
                run(pool)
                done_labels.add(label)

        def ensure(labels, pool):
            while taskq and not all(l in done_labels for l in labels):
                pop_one(pool)

        def pop_toward(labels, pool):
            if taskq and not all(l in done_labels for l in labels):
                pop_one(pool)
            elif taskq and not labels:
                pop_one(pool)

        # ---------------- upfront minimal (own PSUM scope) ----------------
        with tc.tile_pool(name="ups", bufs=4, space="PSUM") as ups:
            ups._pjtag = "pj"
            ups._pjbufs = 4
            # dummy matmuls fill the input-DMA window so the PE HAM clock
            # warms to 2.4 GHz before real work arrives
            wps = ups.tile([128, 128], F32, tag="warm", bufs=2)
            for i in range(64):
                nc.tensor.matmul(
                    out=wps, lhsT=onef, rhs=onef,
                    start=(i == 0), stop=(i == 63), skip_group_check=True)
            for t in range(4):
                v_task(t)[1](ups)
            qk_task("q", 0, 0)[1](ups)
            qk_task("k", 0, 0)[1](ups)
        done_labels.update({("v", t) for t in range(4)})
        done_labels.update({("q", 0, 0), ("k", 0, 0)})

        # remaining work as ordered tasks
        taskq.append(bs_task(0))
        for qc in range(1, NT):
            taskq.append(qk_task("k", 0, qc))
            taskq.append(qk_task("q", 0, qc))
            for t in range(4 * qc, 4 * qc + 4):
                taskq.append(v_task(t))
        for qc in range(NT):
            taskq.append(qk_task("q", 1, qc))
            taskq.append(qk_task("k", 1, qc))
        taskq.append(bs_task(1))
        for hc in range(NP):
            taskq.append(wo_task(hc))
        for pr in range(2, NP):
            for qc in range(NT):
                taskq.append(qk_task("q", pr, qc))
                taskq.append(qk_task("k", pr, qc))
            taskq.append(bs_task(pr))

        # ---------------- attention ----------------
        with tc.tile_pool(name="sps", bufs=2, space="PSUM") as sps, \
             tc.tile_pool(name="pop", bufs=2, space="PSUM") as pop, \
             tc.tile_pool(name="epool", bufs=5) as epool, \
             tc.tile_pool(name="misc", bufs=2) as misc:
            sps._pjtag = "pj"
            sps._pjbufs = 2
            misc_holder["misc"] = misc

            deferred = []   # epilogue-PE + nm/ot closures of previous tile

            def flush_deferred():
                while deferred:
                    deferred.pop(0)()

            def tile_need(pr, t):
                return ([("q", pr, t)] + [("k", pr, kc) for kc in range(t + 1)]
                        + [("v", tb) for tb in range(4 * t + 4)]
                        + [("bs", pr)])

            for pr in range(NP):
                ensure([("q", pr, 0), ("k", pr, 0)]
                       + [("v", t) for t in range(4)], sps)
                QT = qt_tiles[pr]
                KT = kt_tiles[pr]
                for t in range(NT):
                    ensure(tile_need(pr, t), sps)
                    nxt = (tile_need(pr, t + 1) if t + 1 < NT
                           else (tile_need(pr + 1, 0) if pr + 1 < NP else []))
                    q0 = 512 * t
                    chunks = []
                    for kb in range(t):
                        for s2 in range(4):
                            chunks.append((512 * kb + 128 * s2, q0, 512, None))
                    for s2 in range(2):
                        chunks.append((q0 + 128 * s2, q0, 512, ("A", s2)))
                    for s2 in range(2):
                        chunks.append((q0 + 256 + 128 * s2, q0 + 256, 256,
                                       ("B", s2)))

                    po = [pop.tile([65, 512], F32, tag="po", name=f"po{hl}")
                          for hl in range(2)]
                    ntot = len(chunks)
                    pend = []

                    def emit_scores(ci, chunks=chunks, pend=pend, QT=QT, KT=KT):
                        ko, qlo, qn, _ = chunks[ci]
                        pt = sps.tile([128, 2, 512], F32, tag="s", name="pt", bufs=2)
                        for hl in range(2):
                            hs = slice(64 * hl, 64 * (hl + 1))
                            nc.tensor.matmul(
                                out=pt[:, hl, 0:qn],
                                lhsT=KT[hs, ko:ko + 128],
                                rhs=QT[hs, qlo:qlo + qn],
                                start=True, stop=True,
                            )
                        pend.append((ci, pt))

                    def emit_exp(ci, pt, chunks=chunks):
                        # uniform bias -1e-4 scales every softmax term by w
                        # (cancels in the normalization); masked entries then
                        # come out as w (vs exact w^2) -- a 1e-4 relative
                        # perturbation of those weights, far below tolerance.
                        ko, qlo, qn, mr = chunks[ci]
                        if mr is not None:
                            # S' = S*M pre-exp; with the uniform -1e-4 exp
                            # bias, masked lanes come out as w
                            _, s2 = mr
                            nc.vector.tensor_mul(
                                out=pt[:, :, 0:256], in0=pt[:, :, 0:256],
                                in1=mmul_sb[:, s2:s2 + 1, 0:256].broadcast_to(
                                    [128, 2, 256]),
                            )
                        e = epool.tile([128, 2, 512], F32R, tag="e")
                        nc.scalar.activation(
                            out=e[:, :, 0:qn], in_=pt[:, :, 0:qn],
                            func=AF.Exp, scale=0.125, bias=bias_neg[:, 0:1],
                        )
                        return e

                    def emit_pv(ci, e, chunks=chunks, po=po, ntot=ntot,
                                q0=q0, pr=pr):
                        ko, qlo, qn, _ = chunks[ci]
                        qrel = qlo - q0
                        for hl in range(2):
                            nc.tensor.matmul(
                                out=po[hl][:, qrel:qrel + qn],
                                lhsT=V_sb[:, ko // 128, 2 * pr + hl, :],
                                rhs=e[:, hl, 0:qn],
                                start=(ci == 0),
                                stop=(ci == ntot - 1),
                                skip_group_check=True,
                            )

                    # software-pipelined emission (lag 1 chunk); previous
                    # tile's deferred epilogue flushes before our first PV
                    # (its po buffers are about to be reused)
                    epend = []
                    for ci in range(ntot):
                        emit_scores(ci)
                        if len(pend) >= 2:
                            cj, pt = pend.pop(0)
                            epend.append((cj, emit_exp(cj, pt)))
                        if len(epend) >= 2:
                            cj, e = epend.pop(0)
                            if cj == 0:
                                flush_deferred()
                            emit_pv(cj, e)
                            if cj != 0:
                                pop_one(sps)
                    while pend:
                        cj, pt = pend.pop(0)
                        epend.append((cj, emit_exp(cj, pt)))
                    while epend:
                        cj, e = epend.pop(0)
                        if cj == 0:
                            flush_deferred()
                        emit_pv(cj, e)

                    # z-chain now (DVE only); PE parts + nm/ot deferred
                    zrow = misc.tile([1, 1024], F32, tag="zrow", bufs=2)
                    for hl in range(2):
                        nc.vector.scalar_tensor_tensor(
                            out=zrow[:, 512 * hl:512 * (hl + 1)],
                            in0=po[hl][64:65, 0:512], scalar=0.0,
                            in1=zcrow_sb[0:1, 512 * t:512 * (t + 1)],
                            op0=ALU.add, op1=ALU.add,
                        )
                    nc.vector.reciprocal_approx_fast(out=zrow, in_=zrow)
                    zr = misc.tile([1, 1024], F32R, tag="zr", bufs=2)
                    nc.vector.tensor_copy(out=zr, in_=zrow)

                    def late(po=po, zr=zr, pr=pr, t=t, q0=q0):
                        for hl in range(2):
                            zb = sps.tile([64, 512], F32, tag="s", name="zb", bufs=2)
                            nc.tensor.matmul(
                                out=zb, lhsT=ones_fr,
                                rhs=zr[0:1, 512 * hl:512 * (hl + 1)],
                                start=True, stop=True, skip_group_check=True,
                            )
                            nm = misc.tile([64, 512], F32, tag="nm")
                            for qh in range(2):
                                nc.vector.tensor_scalar_add(
                                    out=nm[:, 256 * qh:256 * (qh + 1)],
                                    in0=po[hl][0:64, 256 * qh:256 * (qh + 1)],
                                    scalar1=suf_sb[:, pr, hl,
                                                   2 * t + 1 + qh:2 * t + 2 + qh],
                                )
                            nc.vector.tensor_mul(
                                out=O_sb[64 * hl:64 * (hl + 1), pr, q0:q0 + 512],
                                in0=nm, in1=zb,
                            )
                    deferred.append(late)
                    if pr == NP - 1:
                        # output projection for query chunk t becomes
                        # available once this (last) pair's tile t is done
                        def oq(t=t):
                            for dc in range(8):
                                taskq.append(oproj_task(dc, t))
                        deferred.append(oq)

            flush_deferred()
        with tc.tile_pool(name="fin", bufs=4, space="PSUM") as fin, \
             tc.tile_pool(name="fmisc", bufs=4) as fmisc:
            fin._pjtag = "pj"
            fin._pjbufs = 4
            misc_holder["misc"] = fmisc
            while taskq:
                pop_one(fin)
    nc.compile()
    return nc


def host_in_maps(x, Wqkv, bqkv, Wo, bo):
    import ml_dtypes
    x = np.asarray(x, np.float32)
    Wqkv = np.ascontiguousarray(np.asarray(Wqkv, np.float32))
    bqkv = np.asarray(bqkv, np.float32)
    Wo = np.ascontiguousarray(np.asarray(Wo, np.float32))
    bo = np.asarray(bo, np.float32)

    # triangular 256x256 mask for 2 key sub-blocks, plus all-ones pad
    kap = np.arange(128)[:, None]
    r = np.arange(256)[None, :]
    tri = np.zeros((128, 2, 256), np.float32)
    for s2 in range(2):
        tri[:, s2, :] = (128 * s2 + kap <= r)
    mmul = np.ones((128, 2, 768), np.float32)
    mmul[:, :, 0:256] = tri
    mmul[:, :, 512:768] = tri
    mmul = np.ascontiguousarray(mmul.astype(ml_dtypes.bfloat16))

    wmadd_h = np.ascontiguousarray((1.0 - tri) * W_MASK)

    zcrow = np.zeros((1, S), np.float32)
    for t in range(NT):
        zcrow[0, 512 * t:512 * t + 256] = W_MASK * (S - 512 * t - 256)
        zcrow[0, 512 * t + 256:512 * t + 512] = W_MASK * (S - 512 * t - 512)

    xTs = []
    xsums = []
    for b in range(B):
        xt = np.ascontiguousarray(x[b].T)                # [D, S]
        xTs.append(np.ascontiguousarray(xt.astype(ml_dtypes.bfloat16)))
        xs = xt.reshape(NCH, 128, 8, 256).sum(axis=3)    # [c, p, blk]
        xsums.append(np.ascontiguousarray(
            xs.transpose(1, 0, 2).astype(ml_dtypes.bfloat16)))
    per_p = {}
    for p in range(2):
        cs = slice(512 * p, 512 * p + 512)
        bq = bqkv[0:D][cs]
        bk = bqkv[D:2 * D][cs]
        bv = bqkv[2 * D:][cs]
        wo_p = np.ascontiguousarray(Wo[cs, :])
        bqk = np.zeros((128, 8), np.float32)
        for pr in range(NP):
            bqk[:, pr] = bq[128 * pr:128 * (pr + 1)]
            bqk[:, 4 + pr] = bk[128 * pr:128 * (pr + 1)]
        boc = bv @ wo_p + (bo if p == 0 else 0.0)
        bocol = np.ascontiguousarray(boc.reshape(8, 128).T)
        per_p[p] = {
            "wq": np.ascontiguousarray(Wqkv[:, cs].astype(ml_dtypes.bfloat16)),
            "wk": np.ascontiguousarray(
                Wqkv[:, D + 512 * p:D + 512 * p + 512].astype(ml_dtypes.bfloat16)),
            "wv": np.ascontiguousarray(
                Wqkv[:, 2 * D + 512 * p:2 * D + 512 * p + 512].astype(
                    ml_dtypes.bfloat16)),
            "wo": wo_p,
            "bqk": bqk,
            "bocol": bocol,
        }

    in_maps = []
    for core in range(8):
        b, p = core // 2, core % 2
        m = {"xT": xTs[b], "xsum": xsums[b], "mmul": mmul, "zcrow": zcrow, "wmadd": wmadd_h}
        m.update(per_p[p])
        in_maps.append(m)
    return in_maps


def assemble(results):
    out = np.zeros((B, S, D), np.float32)
    for b in range(B):
        out[b] = (results[2 * b]["outT"] + results[2 * b + 1]["outT"]).T
    return out


_CACHED = {}


def get_program():
    if "nc" not in _CACHED:
        _CACHED["nc"] = build_program()
    return _CACHED["nc"]


def kernel(x, Wqkv, bqkv, Wo, bo):
    from concourse.bass_utils import run_bass_kernel_spmd

    nc = get_program()
    in_maps = host_in_maps(x, Wqkv, bqkv, Wo, bo)
    res = run_bass_kernel_spmd(nc, in_maps, core_ids=list(range(8)))
    return assemble(res.results)
